# revision 1
# baseline (speedup 1.0000x reference)
"""Trainium2 Bass kernel: distance-decay double-softmax attention.

Reference computation per (b, c) pair (L=256, D=512):
    qkv  = x @ w_qkv;  q,k,v = split(qkv)
    attn = softmax(q @ k.T * D_h^-0.5)
    h    = relu((attn + pos) @ w1 + b1);  w = h @ w2 + b2
    attn2= softmax(attn * exp(-dist / (2 w^2 + 1e-6)))
    out  = (attn2 @ v) @ w_out + b_out

Host-side algebraic folds (exact):
    dots = q k^T * s = x (s Wq Wk^T) x^T         -> M = s*Wq@Wk.T
    y    = attn2 @ (v w_out) + b_out             -> Wv' = Wv@w_out
    (attn+pos) @ w1 + b1 = attn@w1 + (pos@w1+b1) -> P1[c] = pos[c]@w1+b1

Sharding: pure data parallel over the 128 (b,c) pairs -> 16 pairs/core,
packed as 8 "superpairs" (2 batch items of one channel share the free
dim, giving N=512 matmuls).  x arrives host-pretransposed; the output
leaves as y^T and is untransposed on the host.  attn / attn2 are
transposed on the PE (via identity).  All matmuls run as float32r
(full-rate fp32 storage) with fp32 PSUM accumulation.

Emission is software-pipelined across superpairs (stage A of superpair
sp is emitted before stage B of superpair sp-1) so the TensorEngine
never drains during the softmax/MLP chain and the HAM clock stays warm.
"""

import sys
import numpy as np

sys.path.insert(0, "/opt/trn_rl_repo")

import concourse.bass as bass  # noqa: E402,F401
import concourse.mybir as mybir  # noqa: E402
from concourse import bacc  # noqa: E402
from concourse.tile import TileContext  # noqa: E402

F32 = mybir.dt.float32
F32R = mybir.dt.float32r
AF = mybir.ActivationFunctionType
ALU = mybir.AluOpType

B, C, L, D = 8, 16, 256, 512
NCORES = 8
CH_PER_CORE = C // NCORES          # 2
NSP = (B // 2) * CH_PER_CORE       # 8 superpairs per core
P = 128
FP = 2 * L                         # 512: two pairs packed along free dim
DT = D // P                        # 4
LT = L // P                        # 2
SCALE = float(64 ** -0.5)          # DIM_HEAD ** -0.5


class _Ctx:
    pass


def _emit_stage_a(g, sp, mid_hook=None, split=0):
    """x load, t^T = (x M)^T, v' = x Wv', dots = t x^T, E=exp(dots)+rowsum.

    split=1 emits only the xt/t/v part; split=2 emits the rest."""
    nc, pp, sp_pool = g.nc, g.pp, g.apool
    MM = nc.tensor.matmul
    if split == 2:
        st = g.state[sp]
        xt, tT = st.xt, st.tT
    else:
        st = g.state[sp] = _Ctx()

        # x^T tiles [128(d), 512(l packed)]
        xt = []
        for dt in range(DT):
            t = sp_pool.tile([P, FP], F32R, tag=f"xt{dt}", name=f"xt{sp}_{dt}")
            nc.sync.dma_start(out=t[:, :],
                              in_=g.h["x_t"][sp, dt * P:(dt + 1) * P, :])
            xt.append(t)
        st.xt = xt

        # t^T[e, l] = sum_d M[d, e] x^T[d, l]
        tT = []
        for et in range(DT):
            ps = pp.tile([P, FP], F32, tag="ps", name=f"ps_t{sp}_{et}")
            for dt in range(DT):
                MM(ps[:, :], g.m_sb[dt][:, et * P:(et + 1) * P], xt[dt][:, :],
                   start=(dt == 0), stop=(dt == DT - 1))
            t = sp_pool.tile([P, FP], F32R, tag=f"tT{et}", name=f"tT{sp}_{et}")
            nc.vector.tensor_copy(t[:, :], ps[:, :])
            tT.append(t)
        st.tT = tT
        if mid_hook is not None:
            mid_hook()

        # v'[l, e] = sum_d x^T[d, l] Wv'[d, e]  (+ b_out fold; per pair)
        v_sb = [[None] * LT for _ in range(2)]
        for pi in range(2):
            for lt in range(LT):
                ps = pp.tile([P, D], F32, tag="ps", name=f"ps_v{sp}_{pi}{lt}")
                for dt in range(DT):
                    MM(ps[:, :],
                       xt[dt][:, pi * L + lt * P: pi * L + (lt + 1) * P],
                       g.wv_sb[dt][:, :],
                       start=(dt == 0), stop=(dt == DT - 1))
                t = sp_pool.tile([P, D], F32R, tag=f"v{pi}{lt}",
                                 name=f"v{sp}_{pi}{lt}")
                nc.vector.tensor_add(t[:, :], ps[:, :], g.bob_sb[:, :])
                v_sb[pi][lt] = t
        st.v = v_sb
    if split == 1:
        return

    # dots[i, m] = sum_e t^T[e, i] x^T[e, m]   (scale folded into M)
    dps = []
    for it in range(LT):
        ps = pp.tile([P, FP], F32, tag="ps", name=f"ps_d{sp}_{it}")
        for pi in range(2):
            o = ps[:, pi * L:(pi + 1) * L]
            for et in range(DT):
                MM(o,
                   tT[et][:, pi * L + it * P: pi * L + (it + 1) * P],
                   xt[et][:, pi * L:(pi + 1) * L],
                   start=(et == 0), stop=(et == DT - 1))
        dps.append(ps)

    # E = exp(dots), s1 = rowsum(E)
    s14 = sp_pool.tile([P, 4], F32, tag="s14", name=f"s14_{sp}")
    E = []
    for it in range(LT):
        e_t = sp_pool.tile([P, FP], F32, tag=f"E{it}", name=f"E{sp}_{it}")
        for pi in range(2):
            c = it * 2 + pi
            sl = slice(pi * L, (pi + 1) * L)
            nc.scalar.activation(e_t[:, sl], dps[it][:, sl], AF.Exp,
                                 accum_out=s14[:, c:c + 1])
        E.append(e_t)
    st.E = E
    r14 = sp_pool.tile([P, 4], F32, tag="r14", name=f"r14_{sp}")
    nc.vector.reciprocal(r14[:, :], s14[:, :])
    st.r14 = r14

    # attn = E * r1  (used by both the MLP transpose and the second softmax)
    attn = []
    for it in range(LT):
        t = sp_pool.tile([P, FP], F32, tag=f"at{it}", name=f"attn{sp}_{it}")
        for pi in range(2):
            c = it * 2 + pi
            sl = slice(pi * L, (pi + 1) * L)
            nc.vector.tensor_scalar_mul(t[:, sl], E[it][:, sl], r14[:, c:c + 1])
        attn.append(t)
    st.attn = attn


def _emit_stage_b1(g, sp):
    """transpose attn, MLP, dist-decay, softmax2 -> attn2 (in wg tiles)."""
    nc, pp, sp_pool = g.nc, g.pp, g.sp_pool
    MM = nc.tensor.matmul
    st = g.state[sp]
    ci = sp // (NSP // CH_PER_CORE)
    attn = st.attn

    # attn^T  [m(part), i(packed free)]
    aT = []
    for mt in range(LT):
        ps = pp.tile([P, FP], F32, tag="ps", name=f"ps_tA{sp}_{mt}")
        for pi in range(2):
            for it in range(LT):
                nc.tensor.transpose(
                    ps[:, pi * L + it * P: pi * L + (it + 1) * P],
                    attn[it][:, pi * L + mt * P: pi * L + (mt + 1) * P],
                    g.id_sb[:, :])
        t = sp_pool.tile([P, FP], F32R, tag=f"trT{mt}", name=f"aT{sp}_{mt}")
        nc.vector.tensor_copy(t[:, :], ps[:, :])
        aT.append(t)

    # h^T = relu(w1^T attn^T + P1^T)   [j(part), i(packed)]
    hT = []
    for jt in range(LT):
        ps = pp.tile([P, FP], F32, tag="ps", name=f"ps_h{sp}_{jt}")
        for mt in range(LT):
            MM(ps[:, :], g.w1_sb[mt][:, jt * P:(jt + 1) * P], aT[mt][:, :],
               start=(mt == 0), stop=False)
        MM(ps[:, :], g.idr_sb[:, :], g.p1_sb[ci][jt][:, :],
           start=False, stop=True)
        t = sp_pool.tile([P, FP], F32R, tag=f"hT{jt}", name=f"hT{sp}_{jt}")
        nc.vector.tensor_scalar_max(t[:, :], ps[:, :], 0.0)
        hT.append(t)

    # w[i] = h[i, :] @ w2 ; negt = -1/(2(w+b2)^2 + 1e-6)
    wps = pp.tile([P, 8], F32, tag="ps", name=f"ps_w{sp}")
    for pi in range(2):
        for it in range(LT):
            c = it * 2 + pi
            for jt in range(LT):
                MM(wps[:, 2 * c:2 * c + 2],
                   hT[jt][:, pi * L + it * P: pi * L + (it + 1) * P],
                   g.w2_sb[jt][:, :],
                   start=(jt == 0), stop=(jt == LT - 1))
    w4 = sp_pool.tile([P, 8], F32, tag="w4", name=f"w4_{sp}")
    nc.scalar.activation(w4[:, :], wps[:, :], AF.Square, bias=g.b2_sb[:, 0:1])
    nc.vector.tensor_scalar(w4[:, :], w4[:, :], -2.0, -1e-6, ALU.mult, ALU.add)
    negt = sp_pool.tile([P, 8], F32, tag="negt", name=f"negt_{sp}")
    nc.vector.reciprocal(negt[:, :], w4[:, :])

    # wg = exp(dist * negt); p2 = attn*wg; E2 = exp(p2) (+s2); attn2 = E2*r2
    s24 = sp_pool.tile([P, 4], F32, tag="s24", name=f"s24_{sp}")
    wg = []
    for it in range(LT):
        t = sp_pool.tile([P, FP], F32, tag=f"wg{it}", name=f"wg{sp}_{it}")
        for pi in range(2):
            c = it * 2 + pi
            sl = slice(pi * L, (pi + 1) * L)
            nc.scalar.activation(t[:, sl], g.dist_sb[it][:, sl], AF.Exp,
                                 scale=negt[:, 2 * c:2 * c + 1])
        nc.vector.tensor_mul(t[:, :], st.attn[it][:, :], t[:, :])
        for pi in range(2):
            c = it * 2 + pi
            sl = slice(pi * L, (pi + 1) * L)
            nc.scalar.activation(t[:, sl], t[:, sl], AF.Exp,
                                 accum_out=s24[:, c:c + 1])
        wg.append(t)
    r24 = sp_pool.tile([P, 4], F32, tag="r24", name=f"r24_{sp}")
    nc.vector.reciprocal(r24[:, :], s24[:, :])
    st.r24 = r24
    st.wg = wg


def _emit_stage_b2(g, sp):
    """transpose E2, y = r2 * (E2 @ v'') in natural row layout, DMA out."""
    nc, pp, sp_pool = g.nc, g.pp, g.sp_pool
    MM = nc.tensor.matmul
    st = g.state[sp]
    wg = st.wg

    # E2^T [m(part), i(packed)]
    a2T = []
    for mt in range(LT):
        ps = pp.tile([P, FP], F32, tag="ps", name=f"ps_tB{sp}_{mt}")
        for pi in range(2):
            for it in range(LT):
                nc.tensor.transpose(
                    ps[:, pi * L + it * P: pi * L + (it + 1) * P],
                    wg[it][:, pi * L + mt * P: pi * L + (mt + 1) * P],
                    g.id_sb[:, :])
        t = sp_pool.tile([P, FP], F32R, tag=f"trT{mt}", name=f"a2T{sp}_{mt}")
        nc.vector.tensor_copy(t[:, :], ps[:, :])
        a2T.append(t)

    # y[i, d'] = r2[i] * sum_m E2^T[m, i] v''[m, d']   (rows sum to 1 after
    # the r2 scale, so b_out folded into v'' is exact)
    for pi in range(2):
        for it in range(LT):
            c = it * 2 + pi
            ps = pp.tile([P, FP], F32, tag="ps", name=f"ps_y{sp}_{pi}{it}")
            for mt in range(LT):
                MM(ps[:, :],
                   a2T[mt][:, pi * L + it * P: pi * L + (it + 1) * P],
                   st.v[pi][mt][:, :],
                   start=(mt == 0), stop=(mt == LT - 1))
            yt = g.ypool.tile([P, FP], F32, tag=f"y{pi}{it}",
                              name=f"y{sp}_{pi}{it}")
            nc.scalar.activation(yt[:, :], ps[:, :], AF.Copy,
                                 scale=st.r24[:, c:c + 1])
            eng = nc.sync if (pi + it) % 2 == 0 else nc.scalar
            eng.dma_start(
                out=g.h["out"][sp, pi * L + it * P: pi * L + (it + 1) * P, :],
                in_=yt[:, :])


def _emit(nc, tc, h):
    import contextlib
    g = _Ctx()
    g.nc, g.h = nc, h
    g.state = {}

    with contextlib.ExitStack() as ex:
        cpool = ex.enter_context(tc.tile_pool(name="consts", bufs=1))
        g.apool = ex.enter_context(tc.tile_pool(name="astream", bufs=3))
        g.sp_pool = ex.enter_context(tc.tile_pool(name="stream", bufs=2))
        g.ypool = ex.enter_context(tc.tile_pool(name="yout", bufs=1))
        g.pp = ex.enter_context(tc.tile_pool(name="ps", bufs=8, space="PSUM"))

        # ---- constants ----
        def cload(name, shape, dt_, src):
            t = cpool.tile(shape, dt_, tag=name, name=name)
            nc.sync.dma_start(out=t[:shape[0], :], in_=src)
            return t

        # Identity first, then ~20 dummy matmuls during the input-DMA head:
        # the HAM clock gate needs ~3.4us of sustained PE activity to lift
        # the 1.2GHz cold throttle, so warm it up while the PE would idle.
        g.idr_sb = cload("identr", [P, P], F32R, h["identr"][:, :])
        warm_ps = g.pp.tile([P, P], F32, tag="ps", name="warmup_ps")
        for wi in range(40):
            nc.tensor.matmul(warm_ps[:, :], g.idr_sb[:, :], g.idr_sb[:, :],
                             start=True, stop=True)

        # Stage-A consts first so the PE can start as soon as m/xt land;
        # everything stage-B needs streams in behind the first A stages.
        g.m_sb = [cload(f"m{dt}", [P, D], F32R, h["m"][dt * P:(dt + 1) * P, :])
                  for dt in range(DT)]
        g.bob_sb = cload("boutb", [P, D], F32R, h["boutb"][:, :])

        def late_consts():
            g.w1_sb = [cload(f"w1_{mt}", [P, L], F32R,
                             h["w1"][mt * P:(mt + 1) * P, :])
                       for mt in range(LT)]
            g.w2_sb = [cload(f"w2_{jt}", [P, 2], F32R,
                             h["w2d"][jt * P:(jt + 1) * P, :])
                       for jt in range(LT)]
            g.p1_sb = [[cload(f"p1_{ci}_{jt}", [P, FP], F32R,
                              h["p1t"][ci, jt * P:(jt + 1) * P, :])
                        for jt in range(LT)] for ci in range(CH_PER_CORE)]
            g.b2_sb = cload("b2r", [P, 1], F32, h["b2r"][:, :])
            g.id_sb = cload("ident", [P, P], F32, h["ident"][:, :])
            g.dist_sb = [cload(f"dist{it}", [P, FP], F32,
                               h["dist"][it * P:(it + 1) * P, :])
                         for it in range(LT)]

        # ---- software-pipelined superpair loop ----
        # PE stream per period: [B1(sp)] [A(sp+2): 48 independent MMs]
        # [B2(sp)] -- the A block covers the softmax/MLP chain latency so
        # B2's transposes never stall the PE.  The tail (no A left) runs
        # B1(6), B1(7), B2(6), B2(7) so B1(7)'s matmuls cover B2(6)'s chain.
        def load_wv():
            g.wv_sb = [cload(f"wv{dt}", [P, D], F32R,
                             h["wv"][dt * P:(dt + 1) * P, :])
                       for dt in range(DT)]

        _emit_stage_a(g, 0, mid_hook=load_wv)
        late_consts()
        _emit_stage_a(g, 1)
        for sp in range(NSP - 3):
            _emit_stage_b1(g, sp)
            _emit_stage_a(g, sp + 2)
            _emit_stage_b2(g, sp)
        # tail: split A(7) so its dots half covers sp=6's chain, and a small
        # warm filler covers sp=7's chain before the final transposes.
        _emit_stage_b1(g, NSP - 3)
        _emit_stage_a(g, NSP - 1, split=1)          # xt, t, v only
        _emit_stage_b2(g, NSP - 3)
        _emit_stage_b1(g, NSP - 2)
        _emit_stage_a(g, NSP - 1, split=2)          # dots, exp1, attn
        _emit_stage_b1(g, NSP - 1)
        _emit_stage_b2(g, NSP - 2)
        fill_ps = g.pp.tile([P, FP], F32, tag="ps", name="fill_tail")
        for wi in range(8):
            nc.tensor.matmul(fill_ps[:, :], g.idr_sb[:, :], g.m_sb[0][:, :],
                             start=True, stop=True)
        _emit_stage_b2(g, NSP - 1)


def build_nc():
    nc = bacc.Bacc("TRN2", target_bir_lowering=False, debug=False,
                   enable_asserts=False)
    h = {}
    h["x_t"] = nc.declare_dram_parameter("x_t", [NSP, D, FP], F32R, False)
    h["m"] = nc.declare_dram_parameter("m", [D, D], F32R, False)
    h["wv"] = nc.declare_dram_parameter("wv", [D, D], F32R, False)
    h["w1"] = nc.declare_dram_parameter("w1", [L, L], F32R, False)
    h["w2d"] = nc.declare_dram_parameter("w2d", [L, 2], F32R, False)
    h["p1t"] = nc.declare_dram_parameter("p1t", [CH_PER_CORE, L, FP], F32R, False)
    h["dist"] = nc.declare_dram_parameter("dist", [L, FP], F32, False)
    h["boutb"] = nc.declare_dram_parameter("boutb", [P, D], F32R, False)
    h["b2r"] = nc.declare_dram_parameter("b2r", [P, 1], F32, False)
    h["ident"] = nc.declare_dram_parameter("ident", [P, P], F32, False)
    h["identr"] = nc.declare_dram_parameter("identr", [P, P], F32R, False)
    h["out"] = nc.declare_dram_parameter("out", [NSP, FP, D], F32, True)

    with TileContext(nc) as tc:
        _emit(nc, tc, h)
    nc.compile()
    return nc


def make_in_maps(x, w_qkv, pos_emb, w1, b1, w2, b2, w_out, b_out):
    f = lambda a: np.ascontiguousarray(np.asarray(a), dtype=np.float32)
    x, w_qkv, pos_emb = f(x), f(w_qkv), f(pos_emb)
    w1, b1, w2, b2, w_out, b_out = f(w1), f(b1), f(w2), f(b2), f(w_out), f(b_out)

    wq, wk, wv = w_qkv[:, :D], w_qkv[:, D:2 * D], w_qkv[:, 2 * D:]
    m = np.ascontiguousarray((SCALE * (wq.astype(np.float64)
                                       @ wk.astype(np.float64).T))
                             .astype(np.float32))
    wvp = np.ascontiguousarray((wv.astype(np.float64)
                                @ w_out.astype(np.float64)).astype(np.float32))
    # P1[c] = pos[c] @ w1 + b1, transposed [L(j), L(i)] per channel
    p1 = pos_emb[0].astype(np.float64) @ w1.astype(np.float64) + b1
    p1t_single = np.ascontiguousarray(p1.transpose(0, 2, 1).astype(np.float32))
    idx = np.arange(L, dtype=np.float32)
    dist = (idx[None, :] - idx[:, None]) ** 2
    distp = np.ascontiguousarray(np.concatenate([dist, dist], axis=1))
    common = {
        "m": m,
        "wv": wvp,
        "w1": w1,
        "w2d": np.ascontiguousarray(np.concatenate([w2, w2], axis=1)),
        "dist": distp,
        "boutb": np.ascontiguousarray(np.tile(b_out.reshape(1, D), (P, 1))),
        "b2r": np.full((P, 1), b2.reshape(-1)[0], np.float32),
        "ident": np.eye(P, dtype=np.float32),
        "identr": np.eye(P, dtype=np.float32),
    }
    in_maps = []
    for core in range(NCORES):
        x_t = np.empty((NSP, D, FP), np.float32)
        p1t = np.empty((CH_PER_CORE, L, FP), np.float32)
        for ci in range(CH_PER_CORE):
            ch = core * CH_PER_CORE + ci
            p1t[ci, :, :L] = p1t_single[ch]
            p1t[ci, :, L:] = p1t_single[ch]
            for bp in range(B // 2):
                s = ci * (B // 2) + bp
                x_t[s, :, :L] = x[2 * bp, ch].T
                x_t[s, :, L:] = x[2 * bp + 1, ch].T
        mcore = dict(common)
        mcore["x_t"] = x_t
        mcore["p1t"] = np.ascontiguousarray(p1t)
        in_maps.append(mcore)
    return in_maps


def assemble_out(results):
    """results: list (per core) of dicts with 'out' [NSP, FP(i-packed), D]."""
    y = np.empty((B, C, L, D), np.float32)
    for core in range(NCORES):
        o = results[core]["out"]
        for ci in range(CH_PER_CORE):
            ch = core * CH_PER_CORE + ci
            for bp in range(B // 2):
                s = ci * (B // 2) + bp
                y[2 * bp, ch] = o[s, :L, :]
                y[2 * bp + 1, ch] = o[s, L:, :]
    return y


_NC = None
LAST_RESULT = None


def kernel(x, w_qkv, pos_emb, w1, b1, w2, b2, w_out, b_out):
    global _NC, LAST_RESULT
    from concourse.bass_utils import run_bass_kernel_spmd

    if _NC is None:
        _NC = build_nc()
    in_maps = make_in_maps(x, w_qkv, pos_emb, w1, b1, w2, b2, w_out, b_out)
    res = run_bass_kernel_spmd(_NC, in_maps, core_ids=list(range(NCORES)))
    LAST_RESULT = res
    return assemble_out(res.results)



# revision 7
# speedup vs baseline: 1.0936x; 1.0936x over previous
"""Trainium2 Bass kernel: distance-decay double-softmax attention.

Reference computation per (b, c) pair (L=256, D=512):
    qkv  = x @ w_qkv;  q,k,v = split(qkv)
    attn = softmax(q @ k.T * D_h^-0.5)
    h    = relu((attn + pos) @ w1 + b1);  w = h @ w2 + b2
    attn2= softmax(attn * exp(-dist / (2 w^2 + 1e-6)))
    out  = (attn2 @ v) @ w_out + b_out

Host-side algebraic folds (exact):
    dots = q k^T * s = x (s Wq Wk^T) x^T   -> M = s*Wq@Wk.T
    y    = attn2 @ (v w_out) + b_out       -> Wv' = Wv@w_out, b_out on host
    pos streamed raw and added on-device (GpSimd), so no P1 precompute.

Dtype strategy (rel-err budget 2e-2; measured ~3e-3 end-to-end for
all-bf16): bf16 for x/M/Wv'/t/E/attn2/v (PE runs bf16 at the same
1 cyc/row as f32r but with half the LDWEIGHTS cost and half the DMA),
fp8e4m3 + DoubleRow (2x PE rate) for the width-MLP whose effect on the
final output is empirically insensitive to quantization (4e-5).

Engine balance per superpair (2 batch items of one channel packed on
the free dim, FP=512): PE ~26K cyc; PSUM->SBUF copies split DVE/Act;
softmax row-sums as single wide DVE tensor_reduce over [it,pi,m]-packed
tiles; the four SBUF-only elementwise passes (attn=E*r1, +pos, *wg,
*r24) as single wide GpSimd tensor_tensor ops with stride-0 broadcast
APs for the per-row scalars; exps/relu/square on Act.
"""

import sys
import numpy as np

sys.path.insert(0, "/opt/trn_rl_repo")

import concourse.bass as bass  # noqa: E402,F401
import concourse.mybir as mybir  # noqa: E402
from concourse import bacc  # noqa: E402
from concourse.tile import TileContext  # noqa: E402

F32 = mybir.dt.float32
BF16 = mybir.dt.bfloat16
F8 = mybir.dt.float8e4
AF = mybir.ActivationFunctionType
ALU = mybir.AluOpType
DR = mybir.MatmulPerfMode.DoubleRow

B, C, L, D = 8, 16, 256, 512
NCORES = 8
CH_PER_CORE = C // NCORES          # 2
NSP = (B // 2) * CH_PER_CORE       # 8 superpairs per core
P = 128
FP = 2 * L                         # 512: two pairs packed along free dim
FP2 = 2 * FP                       # 1024: both i-tiles packed
DT = D // P                        # 4
LT = L // P                        # 2
SCALE = float(64 ** -0.5)          # DIM_HEAD ** -0.5


class _Ctx:
    pass


def _emit_stage_a(g, sp, mid_hook=None, split=0):
    """x load, t^T = (x M)^T, v = x Wv', dots = t x^T, E=exp(dots), s14.

    split=1 emits only the xt/t/v part; split=2 emits the rest."""
    nc, pp, sp_pool = g.nc, g.pp, g.apool
    MM = nc.tensor.matmul
    if split == 2:
        st = g.state[sp]
        xt, tT = st.xt, st.tT
    else:
        st = g.state[sp] = _Ctx()

        # x^T tiles [128(d), 512(l packed)]
        xt = []
        for dt in range(DT):
            t = sp_pool.tile([P, FP], BF16, tag=f"xt{dt}", name=f"xt{sp}_{dt}")
            nc.sync.dma_start(out=t[:, :],
                              in_=g.h["x_t"][sp, dt * P:(dt + 1) * P, :])
            xt.append(t)
        st.xt = xt

        # t^T[e, l] = sum_d M[d, e] x^T[d, l]
        tT = []
        for et in range(DT):
            ps = pp.tile([P, FP], F32, tag="ps", name=f"ps_t{sp}_{et}")
            for dt in range(DT):
                MM(ps[:, :], g.m_sb[dt][:, et * P:(et + 1) * P], xt[dt][:, :],
                   start=(dt == 0), stop=(dt == DT - 1))
            t = sp_pool.tile([P, FP], BF16, tag=f"tT{et}", name=f"tT{sp}_{et}")
            nc.vector.tensor_copy(t[:, :], ps[:, :])
            tT.append(t)
        st.tT = tT
        if mid_hook is not None:
            mid_hook()

        # v[l, e] = sum_d x^T[d, l] Wv'[d, e]  (b_out is added on the host)
        v_sb = [[None] * LT for _ in range(2)]
        for pi in range(2):
            for lt in range(LT):
                ps = pp.tile([P, D], F32, tag="ps", name=f"ps_v{sp}_{pi}{lt}")
                for dt in range(DT):
                    MM(ps[:, :],
                       xt[dt][:, pi * L + lt * P: pi * L + (lt + 1) * P],
                       g.wv_sb[dt][:, :],
                       start=(dt == 0), stop=(dt == DT - 1))
                t = sp_pool.tile([P, D], BF16, tag=f"v{pi}{lt}",
                                 name=f"v{sp}_{pi}{lt}")
                nc.scalar.activation(t[:, :], ps[:, :], AF.Copy)
                v_sb[pi][lt] = t
        st.v = v_sb
    if split == 1:
        return

    # dots[i, m] = sum_e t^T[e, i] x^T[e, m]   (scale folded into M)
    # E packed [128, (it, pi, m)=1024]; one wide exp per it-half; one
    # wide DVE reduce for all four row-sums.
    E = sp_pool.tile([P, FP2], BF16, tag="E", name=f"E_{sp}")
    for it in range(LT):
        ps = pp.tile([P, FP], F32, tag="ps", name=f"ps_d{sp}_{it}")
        for pi in range(2):
            o = ps[:, pi * L:(pi + 1) * L]
            for et in range(DT):
                MM(o,
                   tT[et][:, pi * L + it * P: pi * L + (it + 1) * P],
                   xt[et][:, pi * L:(pi + 1) * L],
                   start=(et == 0), stop=(et == DT - 1))
        nc.scalar.activation(E[:, it * FP:(it + 1) * FP], ps[:, :], AF.Exp)
    st.E = E
    s14 = sp_pool.tile([P, 4], F32, tag="s14", name=f"s14_{sp}")
    nc.vector.tensor_reduce(
        s14[:, :], E[:, :].rearrange("q (c m) -> q c m", c=4),
        axis=mybir.AxisListType.X, op=ALU.add)
    r14 = sp_pool.tile([P, 4], F32, tag="r14", name=f"r14_{sp}")
    nc.vector.reciprocal(r14[:, :], s14[:, :])
    st.r14 = r14


def _bcast4(r):
    """[P,4] per-(it,pi) scalars -> broadcast AP matching [P,(it,pi,m)]."""
    return r[:, :].unsqueeze(2).broadcast_to((P, 4, L))


def _emit_stage_b1(g, sp):
    """attn/apw on GpSimd, transpose, fp8 MLP -> negt, decay, softmax2."""
    nc, pp, sp_pool = g.nc, g.pp, g.sp_pool
    MM = nc.tensor.matmul
    st = g.state[sp]
    ci = sp // (NSP // CH_PER_CORE)
    E, r14 = st.E, st.r14

    # attn = E * r1;  apw = attn + pos   (GpSimd, single wide ops)
    attn = sp_pool.tile([P, FP2], BF16, tag="attn", name=f"attn_{sp}")
    nc.gpsimd.tensor_mul(attn[:, :], E[:, :], _bcast4(r14))
    st.attn = attn
    apw = sp_pool.tile([P, FP2], BF16, tag="apw", name=f"apw_{sp}")
    pos_b = (g.pos_sb[ci][:, :].rearrange("q (a m) -> q a m", a=2)
             .unsqueeze(2).broadcast_to((P, 2, 2, L)))
    nc.gpsimd.tensor_add(apw[:, :], attn[:, :], pos_b)

    # apw^T -> one bf16 PSUM tile [128, (kk=m-half, i packed)=1024]
    aps = pp.tile([P, FP2], BF16, tag="ps", name=f"ps_tA{sp}")
    for mt in range(LT):
        for pi in range(2):
            for it in range(LT):
                nc.tensor.transpose(
                    aps[:, mt * FP + pi * L + it * P:
                        mt * FP + pi * L + (it + 1) * P],
                    apw[:, it * FP + pi * L + mt * P:
                        it * FP + pi * L + (mt + 1) * P],
                    g.id_sb[:, :])
    aT8 = sp_pool.tile([P, 2, FP], F8, tag="aT8", name=f"aT8_{sp}")
    nc.vector.tensor_copy(aT8[:, :, :], aps[:, :])

    # h^T = relu(w1^T apw^T + b1), fp8 DoubleRow (K=256 in one pass)
    hT8 = sp_pool.tile([P, 2, FP], F8, tag="hT8", name=f"hT8_{sp}")
    for jt in range(LT):
        ps = pp.tile([P, FP], F32, tag="ps", name=f"ps_h{sp}_{jt}")
        MM(ps[:, :], g.w1_sb[:, :, jt * P:(jt + 1) * P], aT8[:, :, :],
           start=True, stop=True, perf_mode=DR)
        nc.scalar.activation(hT8[:, jt, :], ps[:, :], AF.Relu,
                             bias=g.b1_sb[:, jt:jt + 1])

    # w^T[c, i] = sum_j w2[j] h^T[j, i]  (c=0,1 identical), one DR matmul
    wps = pp.tile([P, FP], F32, tag="ps", name=f"ps_w{sp}")
    MM(wps[:, :], g.w2_sb[:, :, :], hT8[:, :, :],
       start=True, stop=True, perf_mode=DR)
    wTs = sp_pool.tile([2, FP], BF16, tag="wTs", name=f"wTs_{sp}")
    nc.vector.tensor_copy(wTs[:, :], wps[0:2, :])

    # transpose w^T back to [i(part), 8] (cols 2c,2c+1 identical)
    wtp = pp.tile([P, 8], BF16, tag="ps", name=f"ps_wt{sp}")
    for pi in range(2):
        for it in range(LT):
            c = it * 2 + pi
            nc.tensor.transpose(
                wtp[:, 2 * c:2 * c + 2],
                wTs[0:2, pi * L + it * P: pi * L + (it + 1) * P],
                g.id_sb[0:2, 0:2])
    w4 = sp_pool.tile([P, 8], F32, tag="w4", name=f"w4_{sp}")
    nc.scalar.activation(w4[:, :], wtp[:, :], AF.Square, bias=g.b2_sb[:, 0:1])
    nc.vector.tensor_scalar(w4[:, :], w4[:, :], -2.0, -1e-6, ALU.mult, ALU.add)
    negt = sp_pool.tile([P, 8], F32, tag="negt", name=f"negt_{sp}")
    nc.vector.reciprocal(negt[:, :], w4[:, :])

    # wg = exp(dist*negt) (Act, per-c scale); p2 = attn*wg (GpSimd);
    # E2 = exp(p2) (Act); s24 (DVE); attn2 = E2*r24 (GpSimd).
    wg = sp_pool.tile([P, FP2], BF16, tag="wg", name=f"wg_{sp}")
    for it in range(LT):
        for pi in range(2):
            c = it * 2 + pi
            sl = slice(it * FP + pi * L, it * FP + (pi + 1) * L)
            nc.scalar.activation(wg[:, sl], g.dist_sb[it][:, :], AF.Exp,
                                 scale=negt[:, 2 * c:2 * c + 1])
    nc.gpsimd.tensor_mul(wg[:, :], st.attn[:, :], wg[:, :])
    for it in range(LT):
        sl = slice(it * FP, (it + 1) * FP)
        nc.scalar.activation(wg[:, sl], wg[:, sl], AF.Exp)
    s24 = sp_pool.tile([P, 4], F32, tag="s24", name=f"s24_{sp}")
    nc.vector.tensor_reduce(
        s24[:, :], wg[:, :].rearrange("q (c m) -> q c m", c=4),
        axis=mybir.AxisListType.X, op=ALU.add)
    r24 = sp_pool.tile([P, 4], F32, tag="r24", name=f"r24_{sp}")
    nc.vector.reciprocal(r24[:, :], s24[:, :])
    nc.gpsimd.tensor_mul(wg[:, :], wg[:, :], _bcast4(r24))
    st.wg = wg  # = attn2 (normalized)


def _emit_stage_b2(g, sp):
    """transpose attn2, y = attn2 @ v, plain copy out, DMA."""
    nc, pp, sp_pool = g.nc, g.pp, g.sp_pool
    MM = nc.tensor.matmul
    st = g.state[sp]
    wg = st.wg

    a2ps = pp.tile([P, FP2], BF16, tag="ps", name=f"ps_tB{sp}")
    for mt in range(LT):
        for pi in range(2):
            for it in range(LT):
                nc.tensor.transpose(
                    a2ps[:, mt * FP + pi * L + it * P:
                         mt * FP + pi * L + (it + 1) * P],
                    wg[:, it * FP + pi * L + mt * P:
                       it * FP + pi * L + (mt + 1) * P],
                    g.id_sb[:, :])
    a2T = sp_pool.tile([P, FP2], BF16, tag="a2T", name=f"a2T_{sp}")
    nc.vector.tensor_copy(a2T[:, :], a2ps[:, :])

    for pi in range(2):
        for it in range(LT):
            ps = pp.tile([P, D], F32, tag="ps", name=f"ps_y{sp}_{pi}{it}")
            for mt in range(LT):
                MM(ps[:, :],
                   a2T[:, mt * FP + pi * L + it * P:
                       mt * FP + pi * L + (it + 1) * P],
                   st.v[pi][mt][:, :],
                   start=(mt == 0), stop=(mt == LT - 1))
            yt = g.ypool.tile([P, D], BF16, tag=f"y{pi}{it}",
                              name=f"y{sp}_{pi}{it}")
            if (pi + it) % 2 == 0:
                nc.vector.tensor_copy(yt[:, :], ps[:, :])
            else:
                nc.scalar.activation(yt[:, :], ps[:, :], AF.Copy)
            eng = nc.sync if (pi + it) % 2 == 0 else nc.scalar
            eng.dma_start(
                out=g.h["out"][sp, pi * L + it * P: pi * L + (it + 1) * P, :],
                in_=yt[:, :])


def _emit(nc, tc, h):
    import contextlib
    g = _Ctx()
    g.nc, g.h = nc, h
    g.state = {}

    with contextlib.ExitStack() as ex:
        cpool = ex.enter_context(tc.tile_pool(name="consts", bufs=1))
        g.apool = ex.enter_context(tc.tile_pool(name="astream", bufs=3))
        g.sp_pool = ex.enter_context(tc.tile_pool(name="stream", bufs=2))
        g.ypool = ex.enter_context(tc.tile_pool(name="yout", bufs=2))
        g.pp = ex.enter_context(tc.tile_pool(name="ps", bufs=8, space="PSUM"))

        # ---- constants ----
        def cload(name, shape, dt_, src):
            t = cpool.tile(shape, dt_, tag=name, name=name)
            t_all = t[:, :, :] if len(shape) == 3 else t[:, :]
            nc.sync.dma_start(out=t_all, in_=src)
            return t

        # Identity first, then warmup matmuls during the input-DMA head:
        # the HAM clock gate needs ~3.4us of sustained PE activity to lift
        # the cold throttle, so warm it up while the PE would idle.
        g.id_sb = cload("identb", [P, P], BF16, h["identb"][:, :])
        warm_ps = g.pp.tile([P, P], F32, tag="ps", name="warmup_ps")
        for wi in range(40):
            nc.tensor.matmul(warm_ps[:, :], g.id_sb[:, :], g.id_sb[:, :],
                             start=True, stop=True)

        # Stage-A consts first so the PE can start as soon as m/xt land.
        g.m_sb = [cload(f"m{dt}", [P, D], BF16, h["m"][dt * P:(dt + 1) * P, :])
                  for dt in range(DT)]

        def late_consts():
            g.w1_sb = cload("w1dr", [P, 2, L], F8, h["w1_dr"][:, :, :])
            g.w2_sb = cload("w2dr", [P, 2, P], F8, h["w2_dr"][:, :, :])
            g.b1_sb = cload("b1r", [P, 2], F32, h["b1r"][:, :])
            g.b2_sb = cload("b2r", [P, 1], F32, h["b2r"][:, :])
            g.pos_sb = [cload(f"pos{ci}", [P, FP], BF16, h["posb"][ci, :, :])
                        for ci in range(CH_PER_CORE)]
            g.dist_sb = [cload(f"dist{it}", [P, L], F32,
                               h["distb"][it, :, :])
                         for it in range(LT)]

        def load_wv():
            g.wv_sb = [cload(f"wv{dt}", [P, D], BF16,
                             h["wv"][dt * P:(dt + 1) * P, :])
                       for dt in range(DT)]

        # ---- software-pipelined superpair loop ----
        # PE stream per period: [B1(sp)] [A(sp+2): independent MMs]
        # [B2(sp)] -- the A block covers the softmax/MLP chain latency so
        # B2's transposes never stall the PE.
        _emit_stage_a(g, 0, mid_hook=load_wv)
        late_consts()
        _emit_stage_a(g, 1)
        for sp in range(NSP - 3):
            _emit_stage_b1(g, sp)
            _emit_stage_a(g, sp + 2)
            _emit_stage_b2(g, sp)
        # tail: split A(7) so its dots half covers sp=6's chain, and a small
        # warm filler covers sp=7's chain before the final transposes.
        _emit_stage_b1(g, NSP - 3)
        _emit_stage_a(g, NSP - 1, split=1)          # xt, t, v only
        _emit_stage_b2(g, NSP - 3)
        _emit_stage_b1(g, NSP - 2)
        _emit_stage_a(g, NSP - 1, split=2)          # dots, exp1, s14
        _emit_stage_b1(g, NSP - 1)
        _emit_stage_b2(g, NSP - 2)
        fill_ps = g.pp.tile([P, FP], F32, tag="ps", name="fill_tail")
        for wi in range(8):
            nc.tensor.matmul(fill_ps[:, :], g.id_sb[:, :], g.m_sb[0][:, :],
                             start=True, stop=True)
        _emit_stage_b2(g, NSP - 1)


def build_nc():
    nc = bacc.Bacc("TRN2", target_bir_lowering=False, debug=False,
                   enable_asserts=False)
    h = {}
    h["x_t"] = nc.declare_dram_parameter("x_t", [NSP, D, FP], BF16, False)
    h["m"] = nc.declare_dram_parameter("m", [D, D], BF16, False)
    h["wv"] = nc.declare_dram_parameter("wv", [D, D], BF16, False)
    h["w1_dr"] = nc.declare_dram_parameter("w1_dr", [P, 2, L], F8, False)
    h["w2_dr"] = nc.declare_dram_parameter("w2_dr", [P, 2, P], F8, False)
    h["b1r"] = nc.declare_dram_parameter("b1r", [P, 2], F32, False)
    h["b2r"] = nc.declare_dram_parameter("b2r", [P, 1], F32, False)
    h["posb"] = nc.declare_dram_parameter(
        "posb", [CH_PER_CORE, P, FP], BF16, False)
    h["distb"] = nc.declare_dram_parameter("distb", [LT, P, L], F32, False)
    h["identb"] = nc.declare_dram_parameter("identb", [P, P], BF16, False)
    h["out"] = nc.declare_dram_parameter("out", [NSP, FP, D], BF16, True)

    with TileContext(nc) as tc:
        _emit(nc, tc, h)
    nc.compile()
    return nc


def make_in_maps(x, w_qkv, pos_emb, w1, b1, w2, b2, w_out, b_out):
    import ml_dtypes
    BFNP = ml_dtypes.bfloat16
    F8NP = mybir.dt.np(F8)
    f = lambda a: np.asarray(a, dtype=np.float32)
    x, w_qkv, pos_emb = f(x), f(w_qkv), f(pos_emb)
    w1, b1, w2, b2, w_out, b_out = f(w1), f(b1), f(w2), f(b2), f(w_out), f(b_out)

    wq, wk, wv = w_qkv[:, :D], w_qkv[:, D:2 * D], w_qkv[:, 2 * D:]
    m = (SCALE * (wq.astype(np.float64) @ wk.astype(np.float64).T)
         ).astype(BFNP)
    wvp = (wv.astype(np.float64) @ w_out.astype(np.float64)).astype(BFNP)

    # DoubleRow-packed MLP weights: [p, kk, j] = w[kk*128+p, j]
    w1_dr = np.ascontiguousarray(
        w1.reshape(2, P, L).transpose(1, 0, 2)).astype(F8NP)
    w2_dr = np.zeros((P, 2, P), np.float32)
    w2_dr[:, :, 0:2] = np.broadcast_to(
        w2.reshape(2, P, 1).transpose(1, 0, 2), (P, 2, 2))
    w2_dr = w2_dr.astype(F8NP)
    b1r = np.ascontiguousarray(b1.reshape(2, P).T)
    b2r = np.full((P, 1), b2.reshape(-1)[0], np.float32)

    # pos per channel: [128, (it, m)=512]
    posb_all = pos_emb[0].reshape(C, LT, P, L).transpose(0, 2, 1, 3)
    posb_all = np.ascontiguousarray(posb_all).reshape(C, P, FP).astype(BFNP)

    idx = np.arange(L, dtype=np.float32)
    dist = (idx[None, :] - idx[:, None]) ** 2
    distb = np.ascontiguousarray(dist.reshape(LT, P, L))

    common = {
        "m": np.ascontiguousarray(m),
        "wv": np.ascontiguousarray(wvp),
        "w1_dr": w1_dr,
        "w2_dr": w2_dr,
        "b1r": b1r,
        "b2r": b2r,
        "distb": distb,
        "identb": np.eye(P, dtype=BFNP),
    }
    xb = x.astype(BFNP)
    in_maps = []
    for core in range(NCORES):
        x_t = np.empty((NSP, D, FP), BFNP)
        posb = np.empty((CH_PER_CORE, P, FP), BFNP)
        for ci in range(CH_PER_CORE):
            ch = core * CH_PER_CORE + ci
            posb[ci] = posb_all[ch]
            for bp in range(B // 2):
                s = ci * (B // 2) + bp
                x_t[s, :, :L] = xb[2 * bp, ch].T
                x_t[s, :, L:] = xb[2 * bp + 1, ch].T
        mcore = dict(common)
        mcore["x_t"] = x_t
        mcore["posb"] = posb
        in_maps.append(mcore)
    return in_maps


def assemble_out(results, b_out=None):
    """results: list (per core) of dicts with 'out' [NSP, FP(i-packed), D]."""
    y = np.empty((B, C, L, D), np.float32)
    for core in range(NCORES):
        o = np.asarray(results[core]["out"], np.float32)
        for ci in range(CH_PER_CORE):
            ch = core * CH_PER_CORE + ci
            for bp in range(B // 2):
                s = ci * (B // 2) + bp
                y[2 * bp, ch] = o[s, :L, :]
                y[2 * bp + 1, ch] = o[s, L:, :]
    if b_out is not None:
        y += np.asarray(b_out, np.float32).reshape(1, 1, 1, D)
    return y


_NC = None
LAST_RESULT = None


def kernel(x, w_qkv, pos_emb, w1, b1, w2, b2, w_out, b_out):
    global _NC, LAST_RESULT
    from concourse.bass_utils import run_bass_kernel_spmd

    if _NC is None:
        _NC = build_nc()
    in_maps = make_in_maps(x, w_qkv, pos_emb, w1, b1, w2, b2, w_out, b_out)
    res = run_bass_kernel_spmd(_NC, in_maps, core_ids=list(range(NCORES)))
    LAST_RESULT = res
    return assemble_out(res.results, b_out=b_out)
